# revision 28
# baseline (speedup 1.0000x reference)
"""RBF Nadaraya-Watson regression kernel for Trainium2, 8-core SPMD.

out = (K @ Ytrain) / (sum_j K + EPS),  K = exp(-||xt - xj||^2 / (2 l^2))

Sharding: Xtest rows split across 8 cores; each core holds full
Xtrain/Ytrain and computes its slice independently (no collectives).

Per-core algorithm (T = Ntest/8 = 1024 test rows):
  a_t = ||xtest_t||^2 (fp32), s = 0.5*exp(-2*theta)
  For each 128-row train chunk j:
    b_j = ||xtrain_j||^2 (fp32)
    G^T[j, t] = sum_d Xtrain[j,d] Xtest[t,d]     (PE, bf16 operands)
    K0^T[j,t] = exp(2s*G - s*b_j)                (ACT, bias per partition)
    numer0^T[y, t] += sum_j [Y | 1][j,y] K0^T[j,t]  (PE, bf16)
  out[t,:] = numer0[t, 0:64] / (numer0[t, 64] + EPS*exp(s*a_t))
The a_t term is folded multiplicatively: mathematically identical to
exp(-s(a+b) + 2s c) / (sum exp(...) + EPS) in real arithmetic.
"""

import sys

try:
    import concourse.bass as bass  # noqa: F401
except ImportError:
    sys.path.insert(0, "/opt/trn_rl_repo")

import numpy as np
import ml_dtypes

import concourse.bass as bass
import concourse.bacc as bacc
import concourse.tile as tile
from concourse import mybir
from concourse.bass_utils import run_bass_kernel_spmd

AF = mybir.ActivationFunctionType
F32 = mybir.dt.float32
BF16 = mybir.dt.bfloat16
FP8 = mybir.dt.float8e4

LNEPS = float(np.log(1e-8))
LNHALF = float(np.log(0.5))


def build(T=1024, NTRAIN=32768, D=256, DY=64, SUPER=8, fp8_gram=True,
          fp8_numer=True, dma_transpose=False, reps=1):
    """Build the per-core Bass module. T = test rows per core.

    dma_transpose: load Xtrain as bf16 (SWDGE cast) and transpose chunks on
    the DMA xbar instead of the PE; gram runs in bf16 (overrides fp8_gram).
    reps > 1 repeats the whole computation (for differential device-time
    measurement); the output is simply rewritten each rep.
    """
    if dma_transpose:
        fp8_gram = False
    assert not (fp8_numer and fp8_gram and dma_transpose)
    assert T % 128 == 0 and NTRAIN % (128 * SUPER) == 0 and D == 256
    NCHUNK = NTRAIN // 128
    TT = T // 128
    # moving-dim segments of <= 512
    segs = [(s, min(s + 512, T)) for s in range(0, T, 512)]
    DYP = DY + 1  # Y columns plus the ones column (denominator)

    nc = bacc.Bacc("TRN2", target_bir_lowering=False, debug=False)
    xtrain_d = nc.dram_tensor("Xtrain", [NTRAIN, D], F32, kind="ExternalInput")
    ytrain_d = nc.dram_tensor("Ytrain", [NTRAIN, DY], F32, kind="ExternalInput")
    xtest_d = nc.dram_tensor("Xtest", [T, D], F32, kind="ExternalInput")
    theta_d = nc.dram_tensor("theta", [1, 1], F32, kind="ExternalInput")
    identb_d = nc.dram_tensor("identb", [128, 128], BF16, kind="ExternalInput")
    identf_d = nc.dram_tensor("identf", [128, 128], F32, kind="ExternalInput")
    out_d = nc.dram_tensor("out", [T, DY], F32, kind="ExternalOutput")

    with tile.TileContext(nc) as tc:
      for _rep in range(reps):
        with (
            tc.tile_pool(name="persist", bufs=1) as persist,
            tc.tile_pool(name="xstage", bufs=2) as xstage,
            tc.tile_pool(name="xc8p", bufs=2) as xc8p,
            tc.tile_pool(name="y8p", bufs=2) as y8p,
            tc.tile_pool(name="xtTp", bufs=3) as xtTp,
            tc.tile_pool(name="k0tp", bufs=3) as k0tp,
            tc.tile_pool(name="sqp", bufs=2) as sqp,
            tc.tile_pool(name="biasp", bufs=4) as biasp,
            tc.tile_pool(name="epi", bufs=2) as epi,
            tc.tile_pool(name="gp_pool", bufs=2, space="PSUM") as gp_pool,
            tc.tile_pool(name="np_pool", bufs=1, space="PSUM") as np_pool,
            tc.tile_pool(name="tp_pool", bufs=2, space="PSUM") as tp_pool,
            tc.tile_pool(name="drp", bufs=3, space="DRAM") as drp,
        ):
            # ---- constants / scalars ----
            identb = persist.tile([128, 128], BF16)
            nc.sync.dma_start(identb[:], identb_d.ap())
            identf = persist.tile([128, 128], F32)
            nc.sync.dma_start(identf[:], identf_d.ap())
            theta = persist.tile([1, 1], F32)
            nc.sync.dma_start(theta[:], theta_d.ap())

            sv = persist.tile([1, 3], F32)
            # sv = [2s, s, -s] with s = 0.5*exp(-2*theta)
            nc.scalar.activation(sv[0:1, 0:1], theta[:], AF.Exp, scale=-2.0)
            nc.vector.tensor_scalar_mul(sv[0:1, 1:2], sv[0:1, 0:1], 0.5)
            nc.vector.tensor_scalar_mul(sv[0:1, 2:3], sv[0:1, 0:1], -0.5)

            ones_row = persist.tile([1, 128], F32)
            nc.vector.memset(ones_row[:], 1.0)
            bc_ps = tp_pool.tile([128, 3], F32, tag="t")
            nc.tensor.matmul(bc_ps[:], lhsT=ones_row[:], rhs=sv[0:1, 0:3])
            sbc = persist.tile([128, 3], F32)
            nc.vector.tensor_copy(sbc[:], bc_ps[:])
            s2_vec = sbc[:, 0:1]   # exp(-2 theta) = 2s, broadcast on partitions
            s_vec = sbc[:, 1:2]    # s
            ms_vec = sbc[:, 2:3]   # -s

            # ---- Xtest: a_t, transposed (bf16 or fp8-paired) copies ----
            gdt = FP8 if fp8_gram else BF16
            if fp8_gram:
                xtestT8 = persist.tile([128, D // 128, T], FP8)
            else:
                xtestT = []
                for k in range(D // 128):
                    xtestT_k = persist.tile([128, T], BF16, name=f"xtestT_{k}")
                    xtestT.append(xtestT_k)
            a8 = persist.tile([128, TT], F32)
            for tt in range(TT):
                xts = xstage.tile([128, D], F32, tag="xts")
                nc.sync.dma_start(xts[:], xtest_d.ap()[tt * 128:(tt + 1) * 128, :])
                sqs = sqp.tile([128, D], F32, tag="sq")
                nc.vector.tensor_mul(sqs[:], xts[:], xts[:])
                nc.vector.reduce_sum(a8[:, tt:tt + 1], sqs[:],
                                     axis=mybir.AxisListType.X)
                for k in range(D // 128):
                    tps = tp_pool.tile([128, 128], F32, tag="t", name="tps")
                    nc.tensor.transpose(tps[:], xts[:, k * 128:(k + 1) * 128], identf[:])
                    dst = (xtestT8[:, k, tt * 128:(tt + 1) * 128] if fp8_gram
                           else xtestT[k][:, tt * 128:(tt + 1) * 128])
                    nc.vector.tensor_copy(dst, tps[:])

            sa8 = persist.tile([128, TT], F32)
            nc.vector.tensor_scalar_mul(sa8[:], a8[:], s_vec)
            lneps_t = persist.tile([128, 1], F32)
            nc.vector.memset(lneps_t[:], LNEPS)
            epst8 = persist.tile([128, TT], F32)
            # EPS * exp(s*a_t) = exp(s*a_t + ln(EPS))
            nc.scalar.activation(epst8[:], sa8[:], AF.Exp, bias=lneps_t[:])

            # ---- main loop over train chunks ----
            np_ps = np_pool.tile([DYP, T], F32)
            nsuper = NCHUNK // SUPER
            for c0 in range(nsuper):
                r0 = c0 * SUPER * 128
                xc8 = xc8p.tile([128, SUPER, D],
                                BF16 if dma_transpose else F32, tag="xc8")
                xc8_src = xtrain_d.ap()[r0:r0 + SUPER * 128, :].rearrange(
                    "(c p) d -> p c d", p=128)
                if dma_transpose:
                    nc.gpsimd.dma_start(xc8[:], xc8_src)  # SWDGE casts to bf16
                else:
                    nc.sync.dma_start(xc8[:], xc8_src)
                # fp8 DoubleRow LDWEIGHTS requires the outermost free step to
                # be even and 16B-aligned, so pad the Y plane stride to 80.
                YSTRIDE = 80 if fp8_numer else DYP
                y8 = y8p.tile([128, SUPER, YSTRIDE], FP8 if fp8_numer else BF16,
                              tag="y8")
                nc.gpsimd.dma_start(
                    y8[:, :, 0:DY],
                    ytrain_d.ap()[r0:r0 + SUPER * 128, :].rearrange(
                        "(c p) y -> p c y", p=128),
                )
                nc.vector.memset(y8[:, :, DY:DYP], 1.0)

                if dma_transpose:
                    # batched ||x_j||^2 for the whole superload (bf16 squares,
                    # fp32 row-sums), then bias8 = -s * b8
                    sq8 = sqp.tile([128, SUPER, D], BF16, tag="sq8",
                                   name="sq8")
                    nc.vector.tensor_mul(sq8[:], xc8[:], xc8[:])
                    b8 = biasp.tile([128, SUPER], F32, tag="b8", name="b8")
                    nc.vector.reduce_sum(b8[:], sq8[:],
                                         axis=mybir.AxisListType.X)
                    bias8 = biasp.tile([128, SUPER], F32, tag="bias8",
                                       name="bias8")
                    nc.vector.tensor_scalar_mul(bias8[:], b8[:], ms_vec)

                    # stage the bf16 superload to DRAM, then transpose the
                    # whole superload per d-chunk on the DMA xbar
                    # ([SUPER*128, 128] -> [128, SUPER*128]).
                    xbf = drp.tile([SUPER * 128, D], BF16, tag="xbf",
                                   name="xbf")
                    nc.sync.dma_start(
                        xbf[:].rearrange("(c p) d -> p c d", p=128), xc8[:])
                    xtT8 = []
                    for k in range(D // 128):
                        xtT8_k = xtTp.tile([128, SUPER * 128], BF16,
                                           tag=f"xtT8_{k}",
                                           name=f"xtT8_{k}")
                        nc.sync.dma_start(
                            xtT8_k[:], xbf[:, k * 128:(k + 1) * 128],
                            transpose=True)
                        xtT8.append(xtT8_k)

                for cc in range(SUPER):
                    c = c0 * SUPER + cc
                    xc = xc8[:, cc, :]
                    if dma_transpose:
                        bias_c = bias8[:, cc:cc + 1]
                    else:
                        sqs = sqp.tile([128, D], F32, tag="sq", name="sqs")
                        bvec = biasp.tile([128, 1], F32, tag="bv", name="bvec")
                        nc.vector.tensor_mul(sqs[:], xc, xc)
                        nc.vector.reduce_sum(bvec[:], sqs[:],
                                             axis=mybir.AxisListType.X)
                        bias_t = biasp.tile([128, 1], F32, tag="bc",
                                            name="bias_t")
                        nc.vector.tensor_scalar_mul(bias_t[:], bvec[:], ms_vec)
                        bias_c = bias_t[:]
                        # transpose the chunk (fp32 in PE, cast on copy-out)
                        tpp = tp_pool.tile([128, 2, 128], F32, tag="t",
                                           name="tpp")
                        for k in range(D // 128):
                            nc.tensor.transpose(
                                tpp[:, k, :], xc[:, k * 128:(k + 1) * 128],
                                identf[:])
                        xtT = xtTp.tile([128, 2, 128], gdt, tag="xtT")
                        nc.vector.tensor_copy(xtT[:], tpp[:])

                    # gram: G^T[j, t] for this chunk
                    gp = gp_pool.tile([128, T], F32, tag="g", name="gp")
                    if fp8_gram:
                        for (s0, s1) in segs:
                            nc.tensor.matmul(
                                gp[:, s0:s1],
                                lhsT=xtT[:],
                                rhs=xtestT8[:, :, s0:s1],
                                perf_mode=mybir.MatmulPerfMode.DoubleRow,
                            )
                    else:
                        for k in range(D // 128):
                            lhsT_k = (xtT8[k][:, cc * 128:(cc + 1) * 128]
                                      if dma_transpose else xtT[:, k, :])
                            for (s0, s1) in segs:
                                nc.tensor.matmul(
                                    gp[:, s0:s1],
                                    lhsT=lhsT_k,
                                    rhs=xtestT[k][:, s0:s1],
                                    start=(k == 0),
                                    stop=(k == D // 128 - 1),
                                    skip_group_check=True,
                                )

                    if fp8_numer:
                        # K0^T = exp(2s*G - s*b_j), fp8; chunk pairs feed one
                        # DoubleRow numer matmul.
                        if cc % 2 == 0:
                            k0t2 = k0tp.tile([128, 2, T], FP8, tag="k0t",
                                             name="k0t2")
                        nc.scalar.activation(
                            k0t2[:, cc % 2, :], gp[:], AF.Exp, bias=bias_c[:],
                            scale=s2_vec)
                        if cc % 2 == 1:
                            for (s0, s1) in segs:
                                nc.tensor.matmul(
                                    np_ps[:, s0:s1],
                                    lhsT=y8[:, cc - 1:cc + 1, 0:DYP],
                                    rhs=k0t2[:, :, s0:s1],
                                    perf_mode=mybir.MatmulPerfMode.DoubleRow,
                                    start=(c == 1),
                                    stop=(c == NCHUNK - 1),
                                    skip_group_check=True,
                                )
                    else:
                        # K0^T = exp(2s*G - s*b_j), bf16
                        k0t = k0tp.tile([128, T], BF16, tag="k0t")
                        nc.scalar.activation(
                            k0t[:], gp[:], AF.Exp, bias=bias_c[:], scale=s2_vec)

                        # numer0^T[y, t] accumulation
                        for (s0, s1) in segs:
                            nc.tensor.matmul(
                                np_ps[:, s0:s1],
                                lhsT=y8[:, cc, :],
                                rhs=k0t[:, s0:s1],
                                start=(c == 0),
                                stop=(c == NCHUNK - 1),
                                skip_group_check=True,
                            )

            # ---- epilogue: transpose numer^T, divide, store ----
            ncopy = epi.tile([DYP, T], F32, bufs=1)
            nc.vector.tensor_copy(ncopy[:], np_ps[:])
            for tt in range(TT):
                ntp = tp_pool.tile([128, DYP], F32, tag="t", name="ntp")
                nc.tensor.transpose(
                    ntp[:], ncopy[:, tt * 128:(tt + 1) * 128], identf[0:DYP, 0:DYP])
                dvec = biasp.tile([128, 1], F32, tag="dv", name="dvec")
                nc.vector.tensor_add(dvec[:], ntp[:, DY:DYP], epst8[:, tt:tt + 1])
                rvec = biasp.tile([128, 1], F32, tag="rv", name="rvec")
                nc.vector.reciprocal(rvec[:], dvec[:])
                otile = epi.tile([128, DY], F32, tag="o", name="otile")
                nc.vector.tensor_scalar_mul(otile[:], ntp[:, 0:DY], rvec[:])
                nc.sync.dma_start(out_d.ap()[tt * 128:(tt + 1) * 128, :], otile[:])

    nc.compile()
    return nc


_NC_CACHE = {}


def _get_nc(T, NTRAIN, D, DY):
    key = (T, NTRAIN, D, DY)
    if key not in _NC_CACHE:
        _NC_CACHE[key] = build(T=T, NTRAIN=NTRAIN, D=D, DY=DY)
    return _NC_CACHE[key]


def make_in_maps(Ytrain, Xtrain, Xtest, log_lengthscale, n_cores=8):
    Xtrain = np.ascontiguousarray(np.asarray(Xtrain, dtype=np.float32))
    Ytrain = np.ascontiguousarray(np.asarray(Ytrain, dtype=np.float32))
    Xtest = np.ascontiguousarray(np.asarray(Xtest, dtype=np.float32))
    theta = np.asarray(log_lengthscale, dtype=np.float32).reshape(1, 1)
    identb = np.eye(128, dtype=ml_dtypes.bfloat16)
    identf = np.eye(128, dtype=np.float32)
    shards = np.split(Xtest, n_cores, axis=0)
    return [
        {
            "Xtrain": Xtrain,
            "Ytrain": Ytrain,
            "Xtest": shards[i],
            "theta": theta,
            "identb": identb,
            "identf": identf,
        }
        for i in range(n_cores)
    ]


def kernel(Ytrain, Xtrain, Xtest, log_lengthscale):
    n_cores = 8
    ntest, d = np.asarray(Xtest).shape
    ntrain, dy = np.asarray(Ytrain).shape
    nc = _get_nc(ntest // n_cores, ntrain, d, dy)
    in_maps = make_in_maps(Ytrain, Xtrain, Xtest, log_lengthscale, n_cores)
    res = run_bass_kernel_spmd(nc, in_maps, core_ids=list(range(n_cores)))
    return np.concatenate([res.results[i]["out"] for i in range(n_cores)], axis=0)


# revision 29
# speedup vs baseline: 1.1141x; 1.1141x over previous
"""RBF Nadaraya-Watson regression kernel for Trainium2, 8-core SPMD.

out = (K @ Ytrain) / (sum_j K + EPS),  K = exp(-||xt - xj||^2 / (2 l^2))

Sharding: Xtest rows split across 8 cores; each core holds full
Xtrain/Ytrain and computes its slice independently (no collectives).

Per-core algorithm (T = Ntest/8 = 1024 test rows):
  a_t = ||xtest_t||^2 (fp32), s = 0.5*exp(-2*theta)
  For each 128-row train chunk j:
    b_j = ||xtrain_j||^2 (fp32)
    G^T[j, t] = sum_d Xtrain[j,d] Xtest[t,d]     (PE, bf16 operands)
    K0^T[j,t] = exp(2s*G - s*b_j)                (ACT, bias per partition)
    numer0^T[y, t] += sum_j [Y | 1][j,y] K0^T[j,t]  (PE, bf16)
  out[t,:] = numer0[t, 0:64] / (numer0[t, 64] + EPS*exp(s*a_t))
The a_t term is folded multiplicatively: mathematically identical to
exp(-s(a+b) + 2s c) / (sum exp(...) + EPS) in real arithmetic.
"""

import sys

try:
    import concourse.bass as bass  # noqa: F401
except ImportError:
    sys.path.insert(0, "/opt/trn_rl_repo")

import numpy as np
import ml_dtypes

import concourse.bass as bass
import concourse.bacc as bacc
import concourse.tile as tile
from concourse import mybir
from concourse.bass_utils import run_bass_kernel_spmd

AF = mybir.ActivationFunctionType
F32 = mybir.dt.float32
BF16 = mybir.dt.bfloat16
FP8 = mybir.dt.float8e4

LNEPS = float(np.log(1e-8))
LNHALF = float(np.log(0.5))


def build(T=1024, NTRAIN=32768, D=256, DY=64, SUPER=8, fp8_gram=True,
          fp8_numer=True, dma_transpose=False, reps=1):
    """Build the per-core Bass module. T = test rows per core.

    dma_transpose: load Xtrain as bf16 (SWDGE cast) and transpose chunks on
    the DMA xbar instead of the PE; gram runs in bf16 (overrides fp8_gram).
    reps > 1 repeats the whole computation (for differential device-time
    measurement); the output is simply rewritten each rep.
    """
    if dma_transpose:
        fp8_gram = False
    assert not (fp8_numer and fp8_gram and dma_transpose)
    assert T % 128 == 0 and NTRAIN % (128 * SUPER) == 0 and D == 256
    NCHUNK = NTRAIN // 128
    TT = T // 128
    # moving-dim segments of <= 512
    segs = [(s, min(s + 512, T)) for s in range(0, T, 512)]
    DYP = DY + 1  # Y columns plus the ones column (denominator)

    nc = bacc.Bacc("TRN2", target_bir_lowering=False, debug=False)
    xtrain_d = nc.dram_tensor("Xtrain", [NTRAIN, D], F32, kind="ExternalInput")
    ytrain_d = nc.dram_tensor("Ytrain", [NTRAIN, DY], F32, kind="ExternalInput")
    xtest_d = nc.dram_tensor("Xtest", [T, D], F32, kind="ExternalInput")
    theta_d = nc.dram_tensor("theta", [1, 1], F32, kind="ExternalInput")
    identb_d = nc.dram_tensor("identb", [128, 128], BF16, kind="ExternalInput")
    identf_d = nc.dram_tensor("identf", [128, 128], F32, kind="ExternalInput")
    out_d = nc.dram_tensor("out", [T, DY], F32, kind="ExternalOutput")

    with tile.TileContext(nc) as tc:
      for _rep in range(reps):
        with (
            tc.tile_pool(name="persist", bufs=1) as persist,
            tc.tile_pool(name="xstage", bufs=2) as xstage,
            tc.tile_pool(name="xc8p", bufs=2) as xc8p,
            tc.tile_pool(name="y8p", bufs=2) as y8p,
            tc.tile_pool(name="xtTp", bufs=4) as xtTp,
            tc.tile_pool(name="k0tp", bufs=4) as k0tp,
            tc.tile_pool(name="sqp", bufs=2) as sqp,
            tc.tile_pool(name="biasp", bufs=4) as biasp,
            tc.tile_pool(name="epi", bufs=2) as epi,
            tc.tile_pool(name="gp_pool", bufs=2, space="PSUM") as gp_pool,
            tc.tile_pool(name="np_pool", bufs=1, space="PSUM") as np_pool,
            tc.tile_pool(name="tp_pool", bufs=2, space="PSUM") as tp_pool,
            tc.tile_pool(name="drp", bufs=3, space="DRAM") as drp,
        ):
            # ---- constants / scalars ----
            identb = persist.tile([128, 128], BF16)
            nc.sync.dma_start(identb[:], identb_d.ap())
            identf = persist.tile([128, 128], F32)
            nc.sync.dma_start(identf[:], identf_d.ap())
            theta = persist.tile([1, 1], F32)
            nc.sync.dma_start(theta[:], theta_d.ap())

            sv = persist.tile([1, 3], F32)
            # sv = [2s, s, -s] with s = 0.5*exp(-2*theta)
            nc.scalar.activation(sv[0:1, 0:1], theta[:], AF.Exp, scale=-2.0)
            nc.vector.tensor_scalar_mul(sv[0:1, 1:2], sv[0:1, 0:1], 0.5)
            nc.vector.tensor_scalar_mul(sv[0:1, 2:3], sv[0:1, 0:1], -0.5)

            ones_row = persist.tile([1, 128], F32)
            nc.vector.memset(ones_row[:], 1.0)
            bc_ps = tp_pool.tile([128, 3], F32, tag="t")
            nc.tensor.matmul(bc_ps[:], lhsT=ones_row[:], rhs=sv[0:1, 0:3])
            sbc = persist.tile([128, 3], F32)
            nc.vector.tensor_copy(sbc[:], bc_ps[:])
            s2_vec = sbc[:, 0:1]   # exp(-2 theta) = 2s, broadcast on partitions
            s_vec = sbc[:, 1:2]    # s
            ms_vec = sbc[:, 2:3]   # -s

            # ---- Xtest: a_t, transposed (bf16 or fp8-paired) copies ----
            gdt = FP8 if fp8_gram else BF16
            if fp8_gram:
                xtestT8 = persist.tile([128, D // 128, T], FP8)
            else:
                xtestT = []
                for k in range(D // 128):
                    xtestT_k = persist.tile([128, T], BF16, name=f"xtestT_{k}")
                    xtestT.append(xtestT_k)
            a8 = persist.tile([128, TT], F32)
            for tt in range(TT):
                xts = xstage.tile([128, D], F32, tag="xts")
                nc.sync.dma_start(xts[:], xtest_d.ap()[tt * 128:(tt + 1) * 128, :])
                sqs = sqp.tile([128, D], F32, tag="sq")
                nc.vector.tensor_mul(sqs[:], xts[:], xts[:])
                nc.vector.reduce_sum(a8[:, tt:tt + 1], sqs[:],
                                     axis=mybir.AxisListType.X)
                for k in range(D // 128):
                    tps = tp_pool.tile([128, 128], F32, tag="t", name="tps")
                    nc.tensor.transpose(tps[:], xts[:, k * 128:(k + 1) * 128], identf[:])
                    dst = (xtestT8[:, k, tt * 128:(tt + 1) * 128] if fp8_gram
                           else xtestT[k][:, tt * 128:(tt + 1) * 128])
                    nc.vector.tensor_copy(dst, tps[:])

            sa8 = persist.tile([128, TT], F32)
            nc.vector.tensor_scalar_mul(sa8[:], a8[:], s_vec)
            lneps_t = persist.tile([128, 1], F32)
            nc.vector.memset(lneps_t[:], LNEPS)
            epst8 = persist.tile([128, TT], F32)
            # EPS * exp(s*a_t) = exp(s*a_t + ln(EPS))
            nc.scalar.activation(epst8[:], sa8[:], AF.Exp, bias=lneps_t[:])

            # ---- main loop over train chunks ----
            np_ps = np_pool.tile([DYP, T], F32)
            nsuper = NCHUNK // SUPER
            for c0 in range(nsuper):
                r0 = c0 * SUPER * 128
                xc8 = xc8p.tile([128, SUPER, D],
                                BF16 if dma_transpose else F32, tag="xc8")
                xc8_src = xtrain_d.ap()[r0:r0 + SUPER * 128, :].rearrange(
                    "(c p) d -> p c d", p=128)
                if dma_transpose:
                    nc.gpsimd.dma_start(xc8[:], xc8_src)  # SWDGE casts to bf16
                else:
                    nc.sync.dma_start(xc8[:], xc8_src)
                # fp8 DoubleRow LDWEIGHTS requires the outermost free step to
                # be even and 16B-aligned, so pad the Y plane stride to 80.
                YSTRIDE = 80 if fp8_numer else DYP
                y8 = y8p.tile([128, SUPER, YSTRIDE], FP8 if fp8_numer else BF16,
                              tag="y8")
                nc.gpsimd.dma_start(
                    y8[:, :, 0:DY],
                    ytrain_d.ap()[r0:r0 + SUPER * 128, :].rearrange(
                        "(c p) y -> p c y", p=128),
                )
                nc.gpsimd.memset(y8[:, :, DY:DYP], 1.0)

                if dma_transpose:
                    # batched ||x_j||^2 for the whole superload (bf16 squares,
                    # fp32 row-sums), then bias8 = -s * b8
                    sq8 = sqp.tile([128, SUPER, D], BF16, tag="sq8",
                                   name="sq8")
                    nc.vector.tensor_mul(sq8[:], xc8[:], xc8[:])
                    b8 = biasp.tile([128, SUPER], F32, tag="b8", name="b8")
                    nc.vector.reduce_sum(b8[:], sq8[:],
                                         axis=mybir.AxisListType.X)
                    bias8 = biasp.tile([128, SUPER], F32, tag="bias8",
                                       name="bias8")
                    nc.vector.tensor_scalar_mul(bias8[:], b8[:], ms_vec)

                    # stage the bf16 superload to DRAM, then transpose the
                    # whole superload per d-chunk on the DMA xbar
                    # ([SUPER*128, 128] -> [128, SUPER*128]).
                    xbf = drp.tile([SUPER * 128, D], BF16, tag="xbf",
                                   name="xbf")
                    nc.sync.dma_start(
                        xbf[:].rearrange("(c p) d -> p c d", p=128), xc8[:])
                    xtT8 = []
                    for k in range(D // 128):
                        xtT8_k = xtTp.tile([128, SUPER * 128], BF16,
                                           tag=f"xtT8_{k}",
                                           name=f"xtT8_{k}")
                        nc.sync.dma_start(
                            xtT8_k[:], xbf[:, k * 128:(k + 1) * 128],
                            transpose=True)
                        xtT8.append(xtT8_k)

                for cc in range(SUPER):
                    c = c0 * SUPER + cc
                    xc = xc8[:, cc, :]
                    if dma_transpose:
                        bias_c = bias8[:, cc:cc + 1]
                    else:
                        sqs = sqp.tile([128, D], F32, tag="sq", name="sqs")
                        bvec = biasp.tile([128, 1], F32, tag="bv", name="bvec")
                        nc.vector.tensor_mul(sqs[:], xc, xc)
                        nc.vector.reduce_sum(bvec[:], sqs[:],
                                             axis=mybir.AxisListType.X)
                        bias_t = biasp.tile([128, 1], F32, tag="bc",
                                            name="bias_t")
                        nc.vector.tensor_scalar_mul(bias_t[:], bvec[:], ms_vec)
                        bias_c = bias_t[:]
                        # transpose the chunk (fp32 in PE, cast on copy-out)
                        tpp = tp_pool.tile([128, 2, 128], F32, tag="t",
                                           name="tpp")
                        for k in range(D // 128):
                            nc.tensor.transpose(
                                tpp[:, k, :], xc[:, k * 128:(k + 1) * 128],
                                identf[:])
                        xtT = xtTp.tile([128, 2, 128], gdt, tag="xtT")
                        nc.vector.tensor_copy(xtT[:], tpp[:])

                    # gram: G^T[j, t] for this chunk
                    gp = gp_pool.tile([128, T], F32, tag="g", name="gp")
                    if fp8_gram:
                        for (s0, s1) in segs:
                            nc.tensor.matmul(
                                gp[:, s0:s1],
                                lhsT=xtT[:],
                                rhs=xtestT8[:, :, s0:s1],
                                perf_mode=mybir.MatmulPerfMode.DoubleRow,
                            )
                    else:
                        for k in range(D // 128):
                            lhsT_k = (xtT8[k][:, cc * 128:(cc + 1) * 128]
                                      if dma_transpose else xtT[:, k, :])
                            for (s0, s1) in segs:
                                nc.tensor.matmul(
                                    gp[:, s0:s1],
                                    lhsT=lhsT_k,
                                    rhs=xtestT[k][:, s0:s1],
                                    start=(k == 0),
                                    stop=(k == D // 128 - 1),
                                    skip_group_check=True,
                                )

                    if fp8_numer:
                        # K0^T = exp(2s*G - s*b_j), fp8; chunk pairs feed one
                        # DoubleRow numer matmul.
                        if cc % 2 == 0:
                            k0t2 = k0tp.tile([128, 2, T], FP8, tag="k0t",
                                             name="k0t2")
                        nc.scalar.activation(
                            k0t2[:, cc % 2, :], gp[:], AF.Exp, bias=bias_c[:],
                            scale=s2_vec)
                        if cc % 2 == 1:
                            for (s0, s1) in segs:
                                nc.tensor.matmul(
                                    np_ps[:, s0:s1],
                                    lhsT=y8[:, cc - 1:cc + 1, 0:DYP],
                                    rhs=k0t2[:, :, s0:s1],
                                    perf_mode=mybir.MatmulPerfMode.DoubleRow,
                                    start=(c == 1),
                                    stop=(c == NCHUNK - 1),
                                    skip_group_check=True,
                                )
                    else:
                        # K0^T = exp(2s*G - s*b_j), bf16
                        k0t = k0tp.tile([128, T], BF16, tag="k0t")
                        nc.scalar.activation(
                            k0t[:], gp[:], AF.Exp, bias=bias_c[:], scale=s2_vec)

                        # numer0^T[y, t] accumulation
                        for (s0, s1) in segs:
                            nc.tensor.matmul(
                                np_ps[:, s0:s1],
                                lhsT=y8[:, cc, :],
                                rhs=k0t[:, s0:s1],
                                start=(c == 0),
                                stop=(c == NCHUNK - 1),
                                skip_group_check=True,
                            )

            # ---- epilogue: transpose numer^T, divide, store ----
            ncopy = epi.tile([DYP, T], F32, bufs=1)
            nc.vector.tensor_copy(ncopy[:], np_ps[:])
            for tt in range(TT):
                ntp = tp_pool.tile([128, DYP], F32, tag="t", name="ntp")
                nc.tensor.transpose(
                    ntp[:], ncopy[:, tt * 128:(tt + 1) * 128], identf[0:DYP, 0:DYP])
                dvec = biasp.tile([128, 1], F32, tag="dv", name="dvec")
                nc.vector.tensor_add(dvec[:], ntp[:, DY:DYP], epst8[:, tt:tt + 1])
                rvec = biasp.tile([128, 1], F32, tag="rv", name="rvec")
                nc.vector.reciprocal(rvec[:], dvec[:])
                otile = epi.tile([128, DY], F32, tag="o", name="otile")
                nc.vector.tensor_scalar_mul(otile[:], ntp[:, 0:DY], rvec[:])
                nc.sync.dma_start(out_d.ap()[tt * 128:(tt + 1) * 128, :], otile[:])

    nc.compile()
    return nc


_NC_CACHE = {}


def _get_nc(T, NTRAIN, D, DY):
    key = (T, NTRAIN, D, DY)
    if key not in _NC_CACHE:
        _NC_CACHE[key] = build(T=T, NTRAIN=NTRAIN, D=D, DY=DY)
    return _NC_CACHE[key]


def make_in_maps(Ytrain, Xtrain, Xtest, log_lengthscale, n_cores=8):
    Xtrain = np.ascontiguousarray(np.asarray(Xtrain, dtype=np.float32))
    Ytrain = np.ascontiguousarray(np.asarray(Ytrain, dtype=np.float32))
    Xtest = np.ascontiguousarray(np.asarray(Xtest, dtype=np.float32))
    theta = np.asarray(log_lengthscale, dtype=np.float32).reshape(1, 1)
    identb = np.eye(128, dtype=ml_dtypes.bfloat16)
    identf = np.eye(128, dtype=np.float32)
    shards = np.split(Xtest, n_cores, axis=0)
    return [
        {
            "Xtrain": Xtrain,
            "Ytrain": Ytrain,
            "Xtest": shards[i],
            "theta": theta,
            "identb": identb,
            "identf": identf,
        }
        for i in range(n_cores)
    ]


def kernel(Ytrain, Xtrain, Xtest, log_lengthscale):
    n_cores = 8
    ntest, d = np.asarray(Xtest).shape
    ntrain, dy = np.asarray(Ytrain).shape
    nc = _get_nc(ntest // n_cores, ntrain, d, dy)
    in_maps = make_in_maps(Ytrain, Xtrain, Xtest, log_lengthscale, n_cores)
    res = run_bass_kernel_spmd(nc, in_maps, core_ids=list(range(n_cores)))
    return np.concatenate([res.results[i]["out"] for i in range(n_cores)], axis=0)
